# revision 2
# baseline (speedup 1.0000x reference)
"""GATv2 (single head) on 8 Trainium2 NeuronCores — instruction-minimized design.

Why: on this runtime every engine instruction costs ~100us regardless of size,
so the kernel is organized around FEW, BIG operations (the baseline's ~2300
per-edge instructions/core dominated its 200-460ms runtime).

Structure:
  - Nodes are dealt round-robin by degree rank to the 8 cores (so all cores
    share one degree-envelope schedule), each core's shard is padded to 2560
    nodes (20 node-tiles of 128; 60 dummies) and degree-sorted, then grouped
    into windows of whole node-tiles. Every node in window w gets exactly d_w
    edge slots (d_w = shared max degree in the window); edge slots are
    SLOT-MAJOR (slot j of all nodes, then slot j+1), so edge (node i, slot j)
    lands at partition i%128 — each node owns a fixed partition and the
    reference's segment softmax / scatter-add become single tensor_reduce
    ops over the slot axis.
  - Phase A: xlr = x @ [Wl|Wr] with an appended ones-row folding (bl+br) into
    the xr half -> local shard [2560, 512] fp16 -> AllGather full table.
  - Phase B per window, 14 fixed instructions + ceil(slots/1024) gathers:
      gather xl[src] slab [128, nt, 256]      (SWDGE, row gather)
      xr window load (plain DMA)
      u = slab + xr_bcast                      [DVE]
      lrelu (scalar_tensor_tensor)             [DVE]
      * att (broadcast row)                    [DVE]
      logits = reduce_f                        [DVE, axis=X]
      exp                                      [ACT]
      * pad mask                               [DVE]
      den = reduce_slots, recip, alpha = wm*r  [3 DVE]
      wslab = slab * alpha_bcast               [DVE]
      msg = reduce_slots, + bias, store        [2 DVE + DMA]
"""

import numpy as np

import concourse.bass as bass
import concourse.bacc as bacc
import concourse.mybir as mybir
import concourse.tile as tile
from concourse import library_config
from concourse.bass_utils import run_bass_kernel_spmd

F16 = mybir.dt.float16
F32 = mybir.dt.float32
I16 = mybir.dt.int16

N = 20000
IN_DIM = 1028
OUT = 256
NEG = 0.2
P = 8
NL = N // P            # 2500 real nodes per core
NLP = 2560             # padded to 20 node-tiles
NTL = NLP // 128       # 20 node-tiles
KP = 1152              # contraction dim padded (1028 feats + ones row + pad)
NKT = KP // 128        # 9 k-tiles
S_TARGET = 6144        # target slots per window
GCAP = 1024            # max indices per dma_gather call


def plan(edge_index):
    """Host-side schedule. Windows are groups of whole 128-node tiles with a
    shared uniform slot count d_w; edge slots are slot-major within a window."""
    src = np.asarray(edge_index[0], dtype=np.int64)
    dst = np.asarray(edge_index[1], dtype=np.int64)
    deg = np.bincount(dst, minlength=N) + 1          # + self loop

    order = np.argsort(deg, kind="stable")
    node_of = np.stack([order[c::P] for c in range(P)])   # [P, NL]
    core_of = np.empty(N, dtype=np.int64)
    lpos_of = np.empty(N, dtype=np.int64)
    for c in range(P):
        core_of[node_of[c]] = c
        lpos_of[node_of[c]] = np.arange(NL)
    row_of = core_of * NLP + lpos_of                 # row in gathered table

    degs = deg[node_of]                              # [P, NL]
    mdeg = np.zeros(NLP, dtype=np.int64)
    mdeg[:NL] = degs.max(axis=0)                     # shared envelope
    mdeg[NL:] = 1                                    # dummies: 1 masked slot

    # windows: greedy equal-slot cut at node-tile granularity
    wins = []                                        # (i0, n_nodes, d_w)
    i = 0
    while i < NLP:
        d = int(mdeg[i:i + 128].max())
        n = 128
        while i + n < NLP:
            d2 = max(d, int(mdeg[i + n:i + n + 128].max()))
            if (n + 128) * d2 > max(S_TARGET, 128 * d2):
                break
            n += 128
            d = d2
        wins.append((i, n, d))
        i += n

    in_edges = [[] for _ in range(N)]
    for s, t in zip(src, dst):
        in_edges[t].append(s)

    tot_slots = sum(n * d for (_, n, d) in wins)
    calls = []                                       # (window, off_idx, n_idx)
    off_slots = {}
    io = 0
    for w, (i0, n, d) in enumerate(wins):
        off_slots[w] = io
        j = 0
        while j < n * d:
            nj = min(GCAP, n * d - j)
            calls.append((w, io + j, nj))
            j += nj
        io += n * d
    tot_idx = io

    gidx = np.zeros((P, 128, tot_idx // 16), dtype=np.int16)
    maskf = np.zeros((P, tot_slots), dtype=np.float16)   # slot-linear mask
    for c in range(P):
        lin = np.zeros(tot_idx, dtype=np.int64)
        for w, (i0, n, d) in enumerate(wins):
            base = off_slots[w]
            for k in range(n):
                li = i0 + k
                if li < NL:
                    g = node_of[c][li]
                    srcs = [g] + in_edges[g]
                    rows = row_of[np.asarray(srcs, dtype=np.int64)]
                else:
                    rows = np.zeros(1, dtype=np.int64)   # dummy node
                # slot-major: slot j of node k at base + j*n + k
                sl = base + np.arange(len(rows)) * n + k
                lin[sl] = rows
                maskf[c, sl] = 1.0
        for (w, j0, nj) in calls:
            blk = lin[j0:j0 + nj].astype(np.int16).reshape(nj // 16, 16).T
            gidx[c, :, j0 // 16:(j0 + nj) // 16] = np.tile(blk, (8, 1))

    # mask in slab layout: slot e of a window at [e%128, e//128]
    mask = np.zeros((P, 128, tot_slots // 128), dtype=np.float16)
    for w, (i0, n, d) in enumerate(wins):
        base = off_slots[w]
        blk = maskf[:, base:base + n * d].reshape(P, n * d // 128, 128)
        mask[:, :, base // 128:(base + n * d) // 128] = blk.transpose(0, 2, 1)

    return wins, calls, off_slots, tot_slots, tot_idx, node_of, gidx, mask


def build_program(wins, calls, off_slots, tot_slots, tot_idx, n_iters=1):
    nc = bacc.Bacc("TRN2", target_bir_lowering=False, debug=False,
                   num_devices=P)

    xT = nc.dram_tensor("xT", [KP, NLP], F16, kind="ExternalInput")
    wlr = nc.dram_tensor("wlr", [KP, 2 * OUT], F16, kind="ExternalInput")
    attb_d = nc.dram_tensor("attb", [128, OUT], F32, kind="ExternalInput")
    biasb_d = nc.dram_tensor("biasb", [128, OUT], F32, kind="ExternalInput")
    gidx_d = nc.dram_tensor("gidx", [128, tot_idx // 16], I16,
                            kind="ExternalInput")
    mask_d = nc.dram_tensor("mask", [128, tot_slots // 128], F16,
                            kind="ExternalInput")
    out_d = nc.dram_tensor("out", [NLP, OUT], F32, kind="ExternalOutput")

    NMT = NLP // 128
    wcalls = {}
    for (w, j0, nj) in calls:
        wcalls.setdefault(w, []).append((j0, nj))

    with tile.TileContext(nc, num_cores=P) as tc:
        with (
            tc.tile_pool(name="dram", bufs=1, space="DRAM") as dram,
            tc.tile_pool(name="const", bufs=1) as cpool,
            tc.tile_pool(name="wA", bufs=2) as wA,
            tc.tile_pool(name="slabp", bufs=1) as slabp,
            tc.tile_pool(name="rot", bufs=2) as rot,
            tc.tile_pool(name="small", bufs=2) as spool,
            tc.tile_pool(name="psA", bufs=2, space="PSUM") as psA,
        ):
            xlr_sh = dram.tile([NLP, 2 * OUT], F16)
            xlr_full = dram.tile([P * NLP, 2 * OUT], F16)

            xT_sl = cpool.tile([128, NKT, NLP], F16)
            nc.sync.dma_start(out=xT_sl[:],
                              in_=xT[:].rearrange("(a p) n -> p a n", p=128))
            wlr_sl = cpool.tile([128, NKT, 2 * OUT], F16)
            nc.sync.dma_start(out=wlr_sl[:],
                              in_=wlr[:].rearrange("(a p) n -> p a n", p=128))
            attb = cpool.tile([128, OUT], F32)
            nc.sync.dma_start(out=attb[:], in_=attb_d[:])
            biasb = cpool.tile([128, OUT], F32)
            nc.sync.dma_start(out=biasb[:], in_=biasb_d[:])
            gidx_t = cpool.tile([128, tot_idx // 16], I16)
            nc.sync.dma_start(out=gidx_t[:], in_=gidx_d[:])
            mask_t = cpool.tile([128, tot_slots // 128], F16)
            nc.sync.dma_start(out=mask_t[:], in_=mask_d[:])
            nc.gpsimd.load_library(library_config.mlp)

            def body():
                # ---- Phase A ----
                for ni in range(NMT):
                    n0 = ni * 128
                    ps = psA.tile([128, 2 * OUT], F32, tag="psA")
                    for k in range(NKT):
                        nc.tensor.matmul(
                            ps[:],
                            lhsT=xT_sl[:, k, n0:n0 + 128],
                            rhs=wlr_sl[:, k, :],
                            start=(k == 0),
                            stop=(k == NKT - 1),
                        )
                    xlr_t = wA.tile([128, 2 * OUT], F16, tag="xlr")
                    nc.vector.tensor_copy(out=xlr_t[:], in_=ps[:])
                    nc.sync.dma_start(out=xlr_sh[n0:n0 + 128, :], in_=xlr_t[:])

                nc.gpsimd.collective_compute(
                    "AllGather",
                    mybir.AluOpType.bypass,
                    replica_groups=[list(range(P))],
                    ins=[xlr_sh.opt()],
                    outs=[xlr_full.opt()],
                )

                # ---- Phase B ----
                for w, (i0, n, d) in enumerate(wins):
                    S = n * d
                    nt = S // 128
                    m = n // 128
                    so = off_slots[w]
                    slab = slabp.tile([128, nt, OUT], F16, tag="slab")
                    for (j0, nj) in wcalls[w]:
                        jl = (j0 - so) // 128
                        nc.gpsimd.dma_gather(
                            out_ap=slab[:, jl:jl + nj // 128, :],
                            in_ap=xlr_full[:, 0:OUT],
                            idxs_ap=gidx_t[:, j0 // 16:(j0 + nj) // 16],
                            num_idxs=nj, num_idxs_reg=nj,
                            elem_size=OUT, elem_step=2 * OUT)
                    xr_t = spool.tile([128, m, OUT], F16, tag="xr")
                    nc.sync.dma_start(
                        out=xr_t[:],
                        in_=xlr_sh[i0:i0 + n, OUT:].rearrange(
                            "(it p) f -> p it f", p=128))

                    u = rot.tile([128, nt, OUT], F16, tag="rot")
                    nc.vector.tensor_tensor(
                        out=u[:].rearrange("p (j it) f -> p j it f", it=m),
                        in0=slab[:].rearrange("p (j it) f -> p j it f", it=m),
                        in1=xr_t[:].rearrange("p (o it) f -> p o it f", o=1)
                            .to_broadcast([128, d, m, OUT]),
                        op=mybir.AluOpType.add)
                    lrs = rot.tile([128, nt, OUT], F16, tag="rot")
                    nc.vector.scalar_tensor_tensor(
                        out=lrs[:], in0=u[:], scalar=NEG, in1=u[:],
                        op0=mybir.AluOpType.mult, op1=mybir.AluOpType.max)
                    lrsS = rot.tile([128, nt, OUT], F16, tag="rot")
                    nc.vector.tensor_tensor(
                        out=lrsS[:], in0=lrs[:],
                        in1=attb[:].rearrange("p (o f) -> p o f", o=1)
                            .to_broadcast([128, nt, OUT]),
                        op=mybir.AluOpType.mult)
                    logit = spool.tile([128, nt], F32, tag="logit")
                    nc.vector.tensor_reduce(
                        out=logit[:], in_=lrsS[:],
                        axis=mybir.AxisListType.X, op=mybir.AluOpType.add)
                    wexp = spool.tile([128, nt], F32, tag="wexp")
                    nc.scalar.activation(
                        wexp[:], logit[:], mybir.ActivationFunctionType.Exp)
                    wm = spool.tile([128, nt], F32, tag="wm")
                    nc.vector.tensor_tensor(
                        out=wm[:], in0=wexp[:],
                        in1=mask_t[:, so // 128:so // 128 + nt],
                        op=mybir.AluOpType.mult)
                    den = spool.tile([128, m], F32, tag="den")
                    nc.vector.tensor_reduce(
                        out=den[:],
                        in_=wm[:].rearrange("p (j it) -> p it j", it=m),
                        axis=mybir.AxisListType.X, op=mybir.AluOpType.add)
                    rcp = spool.tile([128, m], F32, tag="rcp")
                    nc.vector.reciprocal(rcp[:], den[:])
                    alph = spool.tile([128, nt], F16, tag="alph")
                    nc.vector.tensor_tensor(
                        out=alph[:].rearrange("p (j it) -> p j it", it=m),
                        in0=wm[:].rearrange("p (j it) -> p j it", it=m),
                        in1=rcp[:].rearrange("p (o it) -> p o it", o=1)
                            .to_broadcast([128, d, m]),
                        op=mybir.AluOpType.mult)
                    wslab = rot.tile([128, nt, OUT], F16, tag="rot")
                    nc.vector.tensor_tensor(
                        out=wslab[:], in0=slab[:],
                        in1=alph[:].rearrange("p (e o) -> p e o", o=1)
                            .to_broadcast([128, nt, OUT]),
                        op=mybir.AluOpType.mult)
                    msg = spool.tile([128, m, OUT], F32, tag="msg")
                    nc.vector.tensor_reduce(
                        out=msg[:],
                        in_=wslab[:].rearrange("p (j it) f -> p it f j", it=m),
                        axis=mybir.AxisListType.X, op=mybir.AluOpType.add)
                    outw = spool.tile([128, m, OUT], F32, tag="outw")
                    nc.vector.tensor_tensor(
                        out=outw[:], in0=msg[:],
                        in1=biasb[:].rearrange("p (o f) -> p o f", o=1)
                            .to_broadcast([128, m, OUT]),
                        op=mybir.AluOpType.add)
                    nc.sync.dma_start(
                        out=out_d[i0:i0 + n, :].rearrange(
                            "(it p) f -> p it f", p=128),
                        in_=outw[:])

            for _ in range(n_iters):
                body()
    nc.compile()
    return nc


_CACHE = {}


def _get_program(plan_key, wins, calls, off_slots, tot_slots, tot_idx,
                 n_iters):
    key = (plan_key, n_iters)
    if key not in _CACHE:
        _CACHE[key] = build_program(wins, calls, off_slots, tot_slots,
                                    tot_idx, n_iters)
    return _CACHE[key]


def make_in_maps(x, edge_index, Wl, bl, Wr, br, att, bias):
    x = np.asarray(x, dtype=np.float32)
    Wl = np.asarray(Wl, dtype=np.float32)
    Wr = np.asarray(Wr, dtype=np.float32)
    bl = np.asarray(bl, dtype=np.float32)
    br = np.asarray(br, dtype=np.float32)
    att = np.asarray(att, dtype=np.float32)
    bias = np.asarray(bias, dtype=np.float32)

    (wins, calls, off_slots, tot_slots, tot_idx,
     node_of, gidx, mask) = plan(edge_index)

    wlr_h = np.zeros((KP, 2 * OUT), dtype=np.float16)
    wlr_h[:IN_DIM, :OUT] = Wl.astype(np.float16)
    wlr_h[:IN_DIM, OUT:] = Wr.astype(np.float16)
    wlr_h[IN_DIM, OUT:] = (bl + br).astype(np.float16)   # ones-row bias fold
    attb = np.tile(att[None, :], (128, 1)).astype(np.float32)
    biasb = np.tile((bias + bl)[None, :], (128, 1)).astype(np.float32)

    in_maps = []
    for c in range(P):
        xTc = np.zeros((KP, NLP), dtype=np.float16)
        xTc[:IN_DIM, :NL] = x[node_of[c], :].T.astype(np.float16)
        xTc[IN_DIM, :] = 1.0
        in_maps.append({
            "xT": xTc, "wlr": wlr_h, "attb": attb, "biasb": biasb,
            "gidx": gidx[c], "mask": mask[c],
        })
    return (wins, calls, off_slots, tot_slots, tot_idx, node_of, in_maps)


def assemble(results, node_of):
    out = np.empty((N, OUT), dtype=np.float32)
    for c in range(P):
        out[node_of[c], :] = results[c]["out"][:NL, :]
    return out


def kernel(x, edge_index, Wl, bl, Wr, br, att, bias, n_iters=1):
    (wins, calls, off_slots, tot_slots, tot_idx,
     node_of, in_maps) = make_in_maps(x, edge_index, Wl, bl, Wr, br, att, bias)
    plan_key = tuple((i, n, d) for (i, n, d) in wins)
    nc = _get_program(plan_key, wins, calls, off_slots, tot_slots, tot_idx,
                      n_iters)
    res = run_bass_kernel_spmd(nc, in_maps, list(range(P)))
    return assemble(res.results, node_of).astype(np.float32)


# revision 3
# speedup vs baseline: 3.1763x; 3.1763x over previous
"""GATv2 (single head) on 8 Trainium2 NeuronCores — instruction-minimized design.

Why: on this runtime every engine instruction costs ~100us regardless of size,
so the kernel is organized around FEW, BIG operations (the baseline's ~2300
per-edge instructions/core dominated its 200-460ms runtime).

Structure:
  - Nodes are dealt round-robin by degree rank to the 8 cores (so all cores
    share one degree-envelope schedule), each core's shard is padded to 2560
    nodes (20 node-tiles of 128; 60 dummies) and degree-sorted, then grouped
    into windows of whole node-tiles. Every node in window w gets exactly d_w
    edge slots (d_w = shared max degree in the window); edge slots are
    SLOT-MAJOR (slot j of all nodes, then slot j+1), so edge (node i, slot j)
    lands at partition i%128 — each node owns a fixed partition and the
    reference's segment softmax / scatter-add become single tensor_reduce
    ops over the slot axis.
  - Phase A: xlr = x @ [Wl|Wr] with an appended ones-row folding (bl+br) into
    the xr half -> local shard [2560, 512] fp16 -> AllGather full table.
  - Phase B per window, 14 fixed instructions + ceil(slots/1024) gathers:
      gather xl[src] slab [128, nt, 256]      (SWDGE, row gather)
      xr window load (plain DMA)
      u = slab + xr_bcast                      [DVE]
      lrelu (scalar_tensor_tensor)             [DVE]
      * att (broadcast row)                    [DVE]
      logits = reduce_f                        [DVE, axis=X]
      exp                                      [ACT]
      * pad mask                               [DVE]
      den = reduce_slots, recip, alpha = wm*r  [3 DVE]
      wslab = slab * alpha_bcast               [DVE]
      msg = reduce_slots, + bias, store        [2 DVE + DMA]
"""

import numpy as np

import concourse.bass as bass
import concourse.bacc as bacc
import concourse.mybir as mybir
import concourse.tile as tile
from concourse import library_config
from concourse.bass_utils import run_bass_kernel_spmd

F16 = mybir.dt.float16
F32 = mybir.dt.float32
I16 = mybir.dt.int16

N = 20000
IN_DIM = 1028
OUT = 256
NEG = 0.2
P = 8
NL = N // P            # 2500 real nodes per core
NLP = 2560             # padded to 20 node-tiles
NTL = NLP // 128       # 20 node-tiles
KP = 1152              # contraction dim padded (1028 feats + ones row + pad)
NKT = KP // 128        # 9 k-tiles
S_TARGET = 6144        # target slots per window
GCAP = 1024            # max indices per dma_gather call


def plan(edge_index, s_target=None):
    """Host-side schedule. Windows are groups of whole 128-node tiles with a
    shared uniform slot count d_w; edge slots are slot-major within a window."""
    src = np.asarray(edge_index[0], dtype=np.int64)
    dst = np.asarray(edge_index[1], dtype=np.int64)
    deg = np.bincount(dst, minlength=N) + 1          # + self loop

    order = np.argsort(deg, kind="stable")
    node_of = np.stack([order[c::P] for c in range(P)])   # [P, NL]
    core_of = np.empty(N, dtype=np.int64)
    lpos_of = np.empty(N, dtype=np.int64)
    for c in range(P):
        core_of[node_of[c]] = c
        lpos_of[node_of[c]] = np.arange(NL)
    row_of = core_of * NLP + lpos_of                 # row in gathered table

    degs = deg[node_of]                              # [P, NL]
    mdeg = np.zeros(NLP, dtype=np.int64)
    mdeg[:NL] = degs.max(axis=0)                     # shared envelope
    mdeg[NL:] = 1                                    # dummies: 1 masked slot

    # windows: greedy equal-slot cut at node-tile granularity
    wins = []                                        # (i0, n_nodes, d_w)
    i = 0
    while i < NLP:
        d = int(mdeg[i:i + 128].max())
        n = 128
        while i + n < NLP:
            d2 = max(d, int(mdeg[i + n:i + n + 128].max()))
            if (n + 128) * d2 > max(s_target or S_TARGET, 128 * d2):
                break
            n += 128
            d = d2
        wins.append((i, n, d))
        i += n

    in_edges = [[] for _ in range(N)]
    for s, t in zip(src, dst):
        in_edges[t].append(s)

    tot_slots = sum(n * d for (_, n, d) in wins)
    calls = []                                       # (window, off_idx, n_idx)
    off_slots = {}
    io = 0
    for w, (i0, n, d) in enumerate(wins):
        off_slots[w] = io
        j = 0
        while j < n * d:
            nj = min(GCAP, n * d - j)
            calls.append((w, io + j, nj))
            j += nj
        io += n * d
    tot_idx = io

    gidx = np.zeros((P, 128, tot_idx // 16), dtype=np.int16)
    maskf = np.zeros((P, tot_slots), dtype=np.float16)   # slot-linear mask
    for c in range(P):
        lin = np.zeros(tot_idx, dtype=np.int64)
        for w, (i0, n, d) in enumerate(wins):
            base = off_slots[w]
            for k in range(n):
                li = i0 + k
                if li < NL:
                    g = node_of[c][li]
                    srcs = [g] + in_edges[g]
                    rows = row_of[np.asarray(srcs, dtype=np.int64)]
                else:
                    rows = np.zeros(1, dtype=np.int64)   # dummy node
                # slot-major: slot j of node k at base + j*n + k
                sl = base + np.arange(len(rows)) * n + k
                lin[sl] = rows
                maskf[c, sl] = 1.0
        for (w, j0, nj) in calls:
            blk = lin[j0:j0 + nj].astype(np.int16).reshape(nj // 16, 16).T
            gidx[c, :, j0 // 16:(j0 + nj) // 16] = np.tile(blk, (8, 1))

    # mask in slab layout: slot e of a window at [e%128, e//128]
    mask = np.zeros((P, 128, tot_slots // 128), dtype=np.float16)
    for w, (i0, n, d) in enumerate(wins):
        base = off_slots[w]
        blk = maskf[:, base:base + n * d].reshape(P, n * d // 128, 128)
        mask[:, :, base // 128:(base + n * d) // 128] = blk.transpose(0, 2, 1)

    return wins, calls, off_slots, tot_slots, tot_idx, node_of, gidx, mask


def build_program(wins, calls, off_slots, tot_slots, tot_idx, n_iters=1,
                  slab_bufs=1):
    nc = bacc.Bacc("TRN2", target_bir_lowering=False, debug=False,
                   num_devices=P, num_swdge_queues=4)

    xT = nc.dram_tensor("xT", [KP, NLP], F16, kind="ExternalInput")
    wlr = nc.dram_tensor("wlr", [KP, 2 * OUT], F16, kind="ExternalInput")
    attb_d = nc.dram_tensor("attb", [128, OUT], F32, kind="ExternalInput")
    biasb_d = nc.dram_tensor("biasb", [128, OUT], F32, kind="ExternalInput")
    gidx_d = nc.dram_tensor("gidx", [128, tot_idx // 16], I16,
                            kind="ExternalInput")
    mask_d = nc.dram_tensor("mask", [128, tot_slots // 128], F16,
                            kind="ExternalInput")
    out_d = nc.dram_tensor("out", [NLP, OUT], F32, kind="ExternalOutput")

    NMT = NLP // 128
    wcalls = {}
    for (w, j0, nj) in calls:
        wcalls.setdefault(w, []).append((j0, nj))

    with tile.TileContext(nc, num_cores=P) as tc:
        with (
            tc.tile_pool(name="dram", bufs=1, space="DRAM") as dram,
            tc.tile_pool(name="const", bufs=1) as cpool,
            tc.tile_pool(name="wA", bufs=2) as wA,
            tc.tile_pool(name="slabp", bufs=slab_bufs) as slabp,
            tc.tile_pool(name="rot", bufs=2) as rot,
            tc.tile_pool(name="small", bufs=2) as spool,
            tc.tile_pool(name="psA", bufs=2, space="PSUM") as psA,
        ):
            xlr_sh = dram.tile([NLP, 2 * OUT], F16)
            xlr_full = dram.tile([P * NLP, 2 * OUT], F16)

            xT_sl = cpool.tile([128, NKT, NLP], F16)
            nc.sync.dma_start(out=xT_sl[:],
                              in_=xT[:].rearrange("(a p) n -> p a n", p=128))
            wlr_sl = cpool.tile([128, NKT, 2 * OUT], F16)
            nc.sync.dma_start(out=wlr_sl[:],
                              in_=wlr[:].rearrange("(a p) n -> p a n", p=128))
            attb = cpool.tile([128, OUT], F32)
            nc.sync.dma_start(out=attb[:], in_=attb_d[:])
            biasb = cpool.tile([128, OUT], F32)
            nc.sync.dma_start(out=biasb[:], in_=biasb_d[:])
            gidx_t = cpool.tile([128, tot_idx // 16], I16)
            nc.sync.dma_start(out=gidx_t[:], in_=gidx_d[:])
            mask_t = cpool.tile([128, tot_slots // 128], F16)
            nc.sync.dma_start(out=mask_t[:], in_=mask_d[:])
            nc.gpsimd.load_library(library_config.mlp)

            def body():
                # ---- Phase A ----
                for ni in range(NMT):
                    n0 = ni * 128
                    ps = psA.tile([128, 2 * OUT], F32, tag="psA")
                    for k in range(NKT):
                        nc.tensor.matmul(
                            ps[:],
                            lhsT=xT_sl[:, k, n0:n0 + 128],
                            rhs=wlr_sl[:, k, :],
                            start=(k == 0),
                            stop=(k == NKT - 1),
                        )
                    xlr_t = wA.tile([128, 2 * OUT], F16, tag="xlr")
                    nc.vector.tensor_copy(out=xlr_t[:], in_=ps[:])
                    nc.sync.dma_start(out=xlr_sh[n0:n0 + 128, :], in_=xlr_t[:])

                nc.gpsimd.collective_compute(
                    "AllGather",
                    mybir.AluOpType.bypass,
                    replica_groups=[list(range(P))],
                    ins=[xlr_sh.opt()],
                    outs=[xlr_full.opt()],
                )

                # ---- Phase B ----
                for w, (i0, n, d) in enumerate(wins):
                    S = n * d
                    nt = S // 128
                    m = n // 128
                    so = off_slots[w]
                    slab = slabp.tile([128, nt, OUT], F16, tag="slab")
                    for ci, (j0, nj) in enumerate(wcalls[w]):
                        jl = (j0 - so) // 128
                        nc.gpsimd.dma_gather(
                            out_ap=slab[:, jl:jl + nj // 128, :],
                            in_ap=xlr_full[:, 0:OUT],
                            idxs_ap=gidx_t[:, j0 // 16:(j0 + nj) // 16],
                            num_idxs=nj, num_idxs_reg=nj,
                            elem_size=OUT, elem_step=2 * OUT,
                            queue_num=(w * 7 + ci) % 4)
                    xr_t = spool.tile([128, m, OUT], F16, tag="xr")
                    nc.sync.dma_start(
                        out=xr_t[:],
                        in_=xlr_sh[i0:i0 + n, OUT:].rearrange(
                            "(it p) f -> p it f", p=128))

                    u = rot.tile([128, nt, OUT], F16, tag="rot")
                    nc.vector.tensor_tensor(
                        out=u[:].rearrange("p (j it) f -> p j it f", it=m),
                        in0=slab[:].rearrange("p (j it) f -> p j it f", it=m),
                        in1=xr_t[:].rearrange("p (o it) f -> p o it f", o=1)
                            .to_broadcast([128, d, m, OUT]),
                        op=mybir.AluOpType.add)
                    lrs = rot.tile([128, nt, OUT], F16, tag="rot")
                    nc.vector.scalar_tensor_tensor(
                        out=lrs[:], in0=u[:], scalar=NEG, in1=u[:],
                        op0=mybir.AluOpType.mult, op1=mybir.AluOpType.max)
                    lrsS = rot.tile([128, nt, OUT], F16, tag="rot")
                    nc.vector.tensor_tensor(
                        out=lrsS[:], in0=lrs[:],
                        in1=attb[:].rearrange("p (o f) -> p o f", o=1)
                            .to_broadcast([128, nt, OUT]),
                        op=mybir.AluOpType.mult)
                    logit = spool.tile([128, nt], F32, tag="logit")
                    nc.vector.tensor_reduce(
                        out=logit[:], in_=lrsS[:],
                        axis=mybir.AxisListType.X, op=mybir.AluOpType.add)
                    wexp = spool.tile([128, nt], F32, tag="wexp")
                    nc.scalar.activation(
                        wexp[:], logit[:], mybir.ActivationFunctionType.Exp)
                    wm = spool.tile([128, nt], F32, tag="wm")
                    nc.vector.tensor_tensor(
                        out=wm[:], in0=wexp[:],
                        in1=mask_t[:, so // 128:so // 128 + nt],
                        op=mybir.AluOpType.mult)
                    den = spool.tile([128, m], F32, tag="den")
                    nc.vector.tensor_reduce(
                        out=den[:],
                        in_=wm[:].rearrange("p (j it) -> p it j", it=m),
                        axis=mybir.AxisListType.X, op=mybir.AluOpType.add)
                    rcp = spool.tile([128, m], F32, tag="rcp")
                    nc.vector.reciprocal(rcp[:], den[:])
                    alph = spool.tile([128, nt], F16, tag="alph")
                    nc.vector.tensor_tensor(
                        out=alph[:].rearrange("p (j it) -> p j it", it=m),
                        in0=wm[:].rearrange("p (j it) -> p j it", it=m),
                        in1=rcp[:].rearrange("p (o it) -> p o it", o=1)
                            .to_broadcast([128, d, m]),
                        op=mybir.AluOpType.mult)
                    wslab = rot.tile([128, nt, OUT], F16, tag="rot")
                    nc.vector.tensor_tensor(
                        out=wslab[:], in0=slab[:],
                        in1=alph[:].rearrange("p (e o) -> p e o", o=1)
                            .to_broadcast([128, nt, OUT]),
                        op=mybir.AluOpType.mult)
                    msg = spool.tile([128, m, OUT], F32, tag="msg")
                    nc.vector.tensor_reduce(
                        out=msg[:],
                        in_=wslab[:].rearrange("p (j it) f -> p it f j", it=m),
                        axis=mybir.AxisListType.X, op=mybir.AluOpType.add)
                    outw = spool.tile([128, m, OUT], F32, tag="outw")
                    nc.vector.tensor_tensor(
                        out=outw[:], in0=msg[:],
                        in1=biasb[:].rearrange("p (o f) -> p o f", o=1)
                            .to_broadcast([128, m, OUT]),
                        op=mybir.AluOpType.add)
                    nc.sync.dma_start(
                        out=out_d[i0:i0 + n, :].rearrange(
                            "(it p) f -> p it f", p=128),
                        in_=outw[:])

            for _ in range(n_iters):
                body()
    nc.compile()
    return nc


_CACHE = {}


def _get_program(plan_key, wins, calls, off_slots, tot_slots, tot_idx,
                 n_iters):
    key = (plan_key, n_iters)
    if key not in _CACHE:
        _CACHE[key] = build_program(wins, calls, off_slots, tot_slots,
                                    tot_idx, n_iters)
    return _CACHE[key]


def make_in_maps(x, edge_index, Wl, bl, Wr, br, att, bias):
    x = np.asarray(x, dtype=np.float32)
    Wl = np.asarray(Wl, dtype=np.float32)
    Wr = np.asarray(Wr, dtype=np.float32)
    bl = np.asarray(bl, dtype=np.float32)
    br = np.asarray(br, dtype=np.float32)
    att = np.asarray(att, dtype=np.float32)
    bias = np.asarray(bias, dtype=np.float32)

    (wins, calls, off_slots, tot_slots, tot_idx,
     node_of, gidx, mask) = plan(edge_index)

    wlr_h = np.zeros((KP, 2 * OUT), dtype=np.float16)
    wlr_h[:IN_DIM, :OUT] = Wl.astype(np.float16)
    wlr_h[:IN_DIM, OUT:] = Wr.astype(np.float16)
    wlr_h[IN_DIM, OUT:] = (bl + br).astype(np.float16)   # ones-row bias fold
    attb = np.tile(att[None, :], (128, 1)).astype(np.float32)
    biasb = np.tile((bias + bl)[None, :], (128, 1)).astype(np.float32)

    in_maps = []
    for c in range(P):
        xTc = np.zeros((KP, NLP), dtype=np.float16)
        xTc[:IN_DIM, :NL] = x[node_of[c], :].T.astype(np.float16)
        xTc[IN_DIM, :] = 1.0
        in_maps.append({
            "xT": xTc, "wlr": wlr_h, "attb": attb, "biasb": biasb,
            "gidx": gidx[c], "mask": mask[c],
        })
    return (wins, calls, off_slots, tot_slots, tot_idx, node_of, in_maps)


def assemble(results, node_of):
    out = np.empty((N, OUT), dtype=np.float32)
    for c in range(P):
        out[node_of[c], :] = results[c]["out"][:NL, :]
    return out


def kernel(x, edge_index, Wl, bl, Wr, br, att, bias, n_iters=1):
    (wins, calls, off_slots, tot_slots, tot_idx,
     node_of, in_maps) = make_in_maps(x, edge_index, Wl, bl, Wr, br, att, bias)
    plan_key = tuple((i, n, d) for (i, n, d) in wins)
    nc = _get_program(plan_key, wins, calls, off_slots, tot_slots, tot_idx,
                      n_iters)
    res = run_bass_kernel_spmd(nc, in_maps, list(range(P)))
    return assemble(res.results, node_of).astype(np.float32)
